# revision 38
# baseline (speedup 1.0000x reference)
"""Single-head attention (InterModalAttention) Bass kernel for 8 TRN2 cores.

Sharding: batch (4) x query/kv-half (2) -> 8 cores. Core (2b+h) projects
Q/K/V only for its OWN 1024 rows of batch b; the K and V halves are then
exchanged between the pair (2b, 2b+1) with an HBM AllGather so each core
holds the full 2048-key K/V in absolute order. This halves the projection
FLOPs vs computing K/V redundantly per core.

Other design points:
  - bf16 everywhere on the PE (same PE rate as fp32r, half DMA/SBUF);
    accumulation stays fp32 in PSUM.
  - Scores computed TRANSPOSED: scT[j,i] = kT_tile.T @ qT_tile, so the exp'd
    tile is directly the lhsT of the output matmul -- no PE transposes.
  - Softmax row-sums via ones-stationary matmul [1,512] accumulated over
    j-tiles in PSUM; moved to per-partition layout with a DRAM bounce.
  - Single pass over x (2MB per core); first K chunk runs dt-outer/et-inner
    across 8 PSUM banks so the PE starts after ~384KB of DMA.
  - AllGather overlaps with the V/Q projections; readback overlaps with Q.
"""
import sys
import numpy as np

for p in ("/opt/trn_rl_repo",):
    if p not in sys.path:
        sys.path.insert(0, p)

B, S, D = 4, 2048, 1024
NQ = 1024          # queries (and owned keys) per core
NCORES = 8
P = 128
INV_SQRT_D = 1.0 / 32.0
PAIRS = [[0, 1], [2, 3], [4, 5], [6, 7]]

_CACHE = {}


def build_nc():
    from contextlib import ExitStack
    import concourse.mybir as mybir
    import concourse.tile as tile
    from concourse import bacc

    F32 = mybir.dt.float32
    FR = mybir.dt.float32r
    BF = mybir.dt.bfloat16
    AF = mybir.ActivationFunctionType

    nc = bacc.Bacc("TRN2", debug=False, num_devices=NCORES)

    xT = nc.dram_tensor("xT", (D, NQ), BF, kind="ExternalInput")   # own rows only
    wqT = nc.dram_tensor("wqT", (D, D), BF, kind="ExternalInput")
    wkT = nc.dram_tensor("wkT", (D, D), BF, kind="ExternalInput")
    wvT = nc.dram_tensor("wvT", (D, D), BF, kind="ExternalInput")
    bq = nc.dram_tensor("bq", (D,), F32, kind="ExternalInput")
    bk = nc.dram_tensor("bk", (D,), F32, kind="ExternalInput")
    bv = nc.dram_tensor("bv", (D,), F32, kind="ExternalInput")
    out = nc.dram_tensor("out", (NQ, D), F32, kind="ExternalOutput")

    ET = D // P            # 8 e-tiles
    DT = D // P            # 8 d-tiles
    HC = NQ // 512         # 2 s-chunks over own half
    SB = S // P            # 16 j-tiles (full seq)
    HB = NQ // P           # 8 j-tiles (own half)
    IG = NQ // 512         # 2 i-chunks
    EC = D // 512          # 2 e-chunks

    with tile.TileContext(nc) as tc, ExitStack() as ctx:
        consts = ctx.enter_context(tc.tile_pool(name="consts", bufs=1))

        _eng = [nc.sync, nc.scalar]
        _dmac = [0]
        def dma(out_ap, in_ap):
            e = _eng[_dmac[0] % len(_eng)]
            _dmac[0] += 1
            e.dma_start(out_ap, in_ap)

        # resident tensors
        kqv = ctx.enter_context(tc.tile_pool(name="kqv", bufs=1))
        kT_r = [kqv.tile([P, ET, NQ], BF, tag=f"kT{r}", name=f"kT{r}")
                for r in range(2)]
        qT_h = [kqv.tile([P, ET, 512], BF, tag=f"qT{h}", name=f"qT{h}")
                for h in range(IG)]         # [d-part, e-tile, i] per i-chunk
        vN_rr = [[kqv.tile([P, 4, D], BF, tag=f"vN{r}_{s}", name=f"vN{r}_{s}")
                  for s in range(2)] for r in range(2)]
        bv_bcast = consts.tile([P, D], F32)

        # DRAM bounce buffers for the pairwise K/V AllGather
        ccd = ctx.enter_context(tc.tile_pool(name="ccd", bufs=1, space="DRAM"))
        kb_in = [ccd.tile([P, ET, 512], BF, tag=f"kbi{s}", name=f"kbi{s}")
                 for s in range(2)]
        kb_out = [ccd.tile([2, P, ET, 512], BF, tag=f"kbo{s}", name=f"kbo{s}")
                  for s in range(2)]
        vb_in = [ccd.tile([P, 4, D], BF, tag=f"vbi{s}", name=f"vbi{s}")
                 for s in range(2)]
        vb_out = [ccd.tile([2, P, 4, D], BF, tag=f"vbo{s}", name=f"vbo{s}")
                  for s in range(2)]

        # ---- Phase 1: projections over own half, single pass over x ----
        with tc.tile_pool(name="w", bufs=1) as wp, \
             tc.tile_pool(name="xc", bufs=2) as xcp, \
             tc.tile_pool(name="kv_own", bufs=1) as ownp:
            kown_c = [ownp.tile([P, ET, 512], BF, tag=f"ko{c}", name=f"ko{c}")
                      for c in range(HC)]   # [d-part, e-tile, own j] per chunk
            vown_c = [ownp.tile([P, 4, D], BF, tag=f"vo{c}", name=f"vo{c}")
                      for c in range(HC)]   # [own j-part, j-tile, e] per chunk
            wk_sb = wp.tile([P, DT, D], BF)
            wq_sb = wp.tile([P, DT, D], BF)
            wv_sb = wp.tile([P, DT, D], BF)
            # DMA issue in consumption order: wk+x first, then wv, wq
            xc = []
            for hc in range(HC):
                xc.append(xcp.tile([P, DT, 512], BF, tag="xc", name=f"xc{hc}"))
            for dt in range(DT):
                dma(wk_sb[:, dt, :], wkT[dt * P:(dt + 1) * P, :])
                dma(xc[0][:, dt, :], xT[dt * P:(dt + 1) * P, 0:512])

            for dt in range(DT):
                dma(xc[1][:, dt, :], xT[dt * P:(dt + 1) * P, 512:1024])

            # constants (issued after the critical-path DMAs)
            ones_f = consts.tile([1, P], F32)
            nc.vector.memset(ones_f[:], 1.0)
            ones = consts.tile([1, P], FR)
            nc.gpsimd.dma_start(ones[:], ones_f[:])
            onesb = consts.tile([P, 1], BF)
            nc.vector.memset(onesb[:], 1.0)
            bv_sb = consts.tile([1, D], FR)
            nc.gpsimd.dma_start(bv_sb[:], bv[:].rearrange("(one d) -> one d", one=1))
            bq_sb = consts.tile([P, ET], F32)
            nc.scalar.dma_start(bq_sb[:], bq[:].rearrange("(t p) -> p t", p=P))
            bk_sb = consts.tile([P, ET], F32)
            nc.scalar.dma_start(bk_sb[:], bk[:].rearrange("(t p) -> p t", p=P))
            for dt in range(DT):
                dma(wv_sb[:, dt, :], wvT[dt * P:(dt + 1) * P, :])
            for dt in range(DT):
                dma(wq_sb[:, dt, :], wqT[dt * P:(dt + 1) * P, :])

            # chunk-0 K projection dt-outer/et-inner: the first 8 matmuls only
            # need wk[dt0]+x0[dt0], so the PE starts as soon as ~384KB landed.
            with tc.tile_pool(name="p8", bufs=8, space="PSUM") as p8:
                psk0 = [p8.tile([P, 512], F32, tag="p8", name=f"psk0_{et}")
                        for et in range(ET)]
                for dt in range(DT):
                    for et in range(ET):
                        nc.tensor.matmul(psk0[et][:],
                                         wk_sb[:, dt, et * P:(et + 1) * P],
                                         xc[0][:, dt, :], start=(dt == 0),
                                         stop=(dt == DT - 1))
                for et in range(ET):
                    nc.vector.tensor_scalar_add(kown_c[0][:, et, :],
                                                psk0[et][:], bk_sb[:, et:et + 1])

            with tc.tile_pool(name="pp", bufs=3, space="PSUM") as pp:
                # K chunk 0 complete -> bounce + first AllGather piece NOW
                for et in range(ET):
                    dma(kb_in[0][:, et, :], kown_c[0][:, et, :])
                nc.gpsimd.collective_compute(
                    "AllGather", mybir.AluOpType.bypass, replica_groups=PAIRS,
                    ins=[kb_in[0][:].opt()], outs=[kb_out[0][:].opt()])
                for r in range(2):
                    for et in range(ET):
                        dma(kT_r[r][:, et, 0:512], kb_out[0][r, :, et, :])
                # K chunk 1
                for et in range(ET):
                    psk = pp.tile([P, 512], F32, tag="pp")
                    for dt in range(DT):
                        nc.tensor.matmul(psk[:], wk_sb[:, dt, et * P:(et + 1) * P],
                                         xc[1][:, dt, :], start=(dt == 0),
                                         stop=(dt == DT - 1))
                    nc.vector.tensor_scalar_add(kown_c[1][:, et, :],
                                                psk[:], bk_sb[:, et:et + 1])
                # K chunk 1 -> second AllGather piece
                for et in range(ET):
                    dma(kb_in[1][:, et, :], kown_c[1][:, et, :])
                nc.gpsimd.collective_compute(
                    "AllGather", mybir.AluOpType.bypass, replica_groups=PAIRS,
                    ins=[kb_in[1][:].opt()], outs=[kb_out[1][:].opt()])
                for r in range(2):
                    for et in range(ET):
                        dma(kT_r[r][:, et, 512:1024], kb_out[1][r, :, et, :])

                # bv broadcast to [P, D] via ones.T @ bv (K=1 matmul)
                for ec in range(EC):
                    pstmp = pp.tile([P, 512], F32, tag="pp")
                    nc.tensor.matmul(pstmp[:], ones[:],
                                     bv_sb[:, ec * 512:(ec + 1) * 512],
                                     start=True, stop=True)
                    nc.any.tensor_copy(bv_bcast[:, ec * 512:(ec + 1) * 512], pstmp[:])

                # V projection (own half)
                for hc in range(HC):
                    for sb_i in range(4):
                        jg = hc * 4 + sb_i
                        for ec in range(EC):
                            psv = pp.tile([P, 512], F32, tag="pp")
                            for dt in range(DT):
                                nc.tensor.matmul(psv[:],
                                                 xc[hc][:, dt, sb_i * P:(sb_i + 1) * P],
                                                 wv_sb[:, dt, ec * 512:(ec + 1) * 512],
                                                 start=(dt == 0), stop=(dt == DT - 1))
                            nc.any.tensor_copy(
                                vown_c[hc][:, sb_i, ec * 512:(ec + 1) * 512], psv[:])
                # V-half complete -> bounce out + AllGather (overlaps Q)
                for jg in range(HB):
                    dma(vb_in[jg // 4][:, jg % 4, :], vown_c[jg // 4][:, jg % 4, :])
                for s in range(2):
                    nc.gpsimd.collective_compute(
                        "AllGather", mybir.AluOpType.bypass, replica_groups=PAIRS,
                        ins=[vb_in[s][:].opt()], outs=[vb_out[s][:].opt()])
                    for r in range(2):
                        for jg in range(4):
                            dma(vN_rr[r][s][:, jg, :], vb_out[s][r, :, jg, :])

                # Q projection
                for hc in range(HC):
                    for et in range(ET):
                        psq = pp.tile([P, 512], F32, tag="pp")
                        for dt in range(DT):
                            nc.tensor.matmul(psq[:],
                                             wq_sb[:, dt, et * P:(et + 1) * P],
                                             xc[hc][:, dt, :], start=(dt == 0),
                                             stop=(dt == DT - 1))
                        nc.vector.tensor_scalar_add(
                            qT_h[hc][:, et, :],
                            psq[:], bq_sb[:, et:et + 1])

        # ---- Phase 2: attention ----
        # All score chunks first (attn for both i-chunks stays resident), then
        # all output matmuls: pushes the first vN consumer ~30us later so the
        # V AllGather + completion-polling latency is fully hidden. The scores
        # PSUM pools close before the out phase so outps can take all 8 banks,
        # keeping 4 output blocks in flight ahead of the epilogue drains.
        with tc.tile_pool(name="attn", bufs=1) as attnp, \
             tc.tile_pool(name="epi2", bufs=2) as epi2p:
            attnTs = [attnp.tile([P, SB, 512], BF, tag=f"attnT{g}", name=f"attnT{g}")
                      for g in range(IG)]
            invss = []
            with tc.tile_pool(name="scps", bufs=2, space="PSUM") as scps, \
                 tc.tile_pool(name="rsps", bufs=2, space="PSUM") as rsps, \
                 tc.tile_pool(name="rsdram", bufs=2, space="DRAM") as rsdram, \
                 tc.tile_pool(name="epi", bufs=2) as epip:
                for g in range(IG):
                    attnT = attnTs[g]
                    rs = rsps.tile([1, 512], F32, tag="rs", name=f"rs{g}")
                    for jt in range(SB):
                        sc_ps = scps.tile([P, 512], F32, tag="scps")
                        for et in range(ET):
                            nc.tensor.matmul(sc_ps[:],
                                             kT_r[jt // HB][:, et, (jt % HB) * P:(jt % HB + 1) * P],
                                             qT_h[g][:, et, :],
                                             start=(et == 0), stop=(et == ET - 1))
                        nc.scalar.activation(attnT[:, jt, :], sc_ps[:], AF.Exp,
                                             scale=INV_SQRT_D)
                        nc.tensor.matmul(rs[:], onesb[:], attnT[:, jt, :],
                                         start=(jt == 0), stop=(jt == SB - 1))
                    # rowsums -> per-partition [128, 4] via DRAM bounce
                    rs_sb = epip.tile([1, 512], F32, tag="rs_sb")
                    nc.vector.tensor_copy(rs_sb[:], rs[:])
                    rs_d = rsdram.tile([1, 512], F32, tag="rs_d")
                    nc.sync.dma_start(rs_d[:], rs_sb[:])
                    rsT = epip.tile([P, 4], F32, tag="rsT")
                    nc.sync.dma_start(
                        rsT[:], rs_d[:].rearrange("one (b p) -> p (one b)", p=P))
                    invs = epi2p.tile([P, 4], F32, tag="invs", name=f"invs{g}")
                    nc.vector.reciprocal(invs[:], rsT[:])
                    invss.append(invs)
            # output matmuls + epilogue per 128-query block
            with tc.tile_pool(name="outps", bufs=4, space="PSUM") as outps:
                for g in range(IG):
                    attnT = attnTs[g]
                    invs = invss[g]
                    for ib in range(4):
                        i0 = ib * P
                        ops = [outps.tile([P, 512], F32, tag=f"outps{ec}",
                                          name=f"ops{g}_{ib}_{ec}") for ec in range(EC)]
                        for jt in range(SB):
                            for ec in range(EC):
                                nc.tensor.matmul(ops[ec][:],
                                                 attnT[:, jt, i0:i0 + P],
                                                 vN_rr[jt // HB][(jt % HB) // 4][:, jt % 4, ec * 512:(ec + 1) * 512],
                                                 start=(jt == 0), stop=(jt == SB - 1))
                        out_sb = epi2p.tile([P, D], F32, tag="out_sb")
                        # per-half pipeline: ACT normalize -> DVE bias -> DMA,
                        # so the final block's drain chain is ~half as long
                        r0 = g * 512 + i0
                        for ec in range(EC):
                            sl = slice(ec * 512, (ec + 1) * 512)
                            nc.scalar.activation(out_sb[:, sl], ops[ec][:],
                                                 AF.Copy, scale=invs[:, ib:ib + 1])
                            nc.vector.tensor_add(out_sb[:, sl], out_sb[:, sl],
                                                 bv_bcast[:, sl])
                            dma(out[r0:r0 + P, sl], out_sb[:, sl])

    nc.compile()
    return nc


def make_in_maps(x, Wq, bq, Wk, bk, Wv, bv):
    import ml_dtypes
    BF = ml_dtypes.bfloat16
    x = np.asarray(x, np.float32)
    wqT = np.ascontiguousarray(np.asarray(Wq, np.float32).T.astype(BF))
    wkT = np.ascontiguousarray(np.asarray(Wk, np.float32).T.astype(BF))
    wvT = np.ascontiguousarray(np.asarray(Wv, np.float32).T.astype(BF))
    bq = np.ascontiguousarray(np.asarray(bq, np.float32))
    bk = np.ascontiguousarray(np.asarray(bk, np.float32))
    bv = np.ascontiguousarray(np.asarray(bv, np.float32))
    in_maps = []
    for c in range(NCORES):
        b, h = c // 2, c % 2
        xb = x[b]
        in_maps.append({
            "xT": np.ascontiguousarray(xb[h * NQ:(h + 1) * NQ].T.astype(BF)),
            "wqT": wqT, "wkT": wkT, "wvT": wvT,
            "bq": bq, "bk": bk, "bv": bv,
        })
    return in_maps


def get_nc():
    if "nc" not in _CACHE:
        _CACHE["nc"] = build_nc()
    return _CACHE["nc"]


def kernel(x, Wq, bq, Wk, bk, Wv, bv):
    from concourse.bass_utils import run_bass_kernel_spmd
    nc = get_nc()
    in_maps = make_in_maps(x, Wq, bq, Wk, bk, Wv, bv)
    res = run_bass_kernel_spmd(nc, in_maps, core_ids=list(range(NCORES)))
    out = np.empty((B, S, D), np.float32)
    for c in range(NCORES):
        b, h = c // 2, c % 2
        out[b, h * NQ:(h + 1) * NQ] = res.results[c]["out"]
    return out


# revision 42
# speedup vs baseline: 1.1009x; 1.1009x over previous
"""Single-head attention (InterModalAttention) Bass kernel for 8 TRN2 cores.

Sharding: batch (4) x query/kv-half (2) -> 8 cores. Core (2b+h) projects
Q/K/V only for its OWN 1024 rows of batch b; the K and V halves are then
exchanged between the pair (2b, 2b+1) with an HBM AllGather so each core
holds the full 2048-key K/V in absolute order. This halves the projection
FLOPs vs computing K/V redundantly per core.

Other design points:
  - bf16 everywhere on the PE (same PE rate as fp32r, half DMA/SBUF);
    accumulation stays fp32 in PSUM.
  - Scores computed TRANSPOSED: scT[j,i] = kT_tile.T @ qT_tile, so the exp'd
    tile is directly the lhsT of the output matmul -- no PE transposes.
  - Softmax row-sums via ones-stationary matmul [1,512] accumulated over
    j-tiles in PSUM; moved to per-partition layout with a DRAM bounce.
  - Single pass over x (2MB per core); first K chunk runs dt-outer/et-inner
    across 8 PSUM banks so the PE starts after ~384KB of DMA.
  - AllGather overlaps with the V/Q projections; readback overlaps with Q.
"""
import sys
import numpy as np

for p in ("/opt/trn_rl_repo",):
    if p not in sys.path:
        sys.path.insert(0, p)

B, S, D = 4, 2048, 1024
NQ = 1024          # queries (and owned keys) per core
NCORES = 8
P = 128
INV_SQRT_D = 1.0 / 32.0
PAIRS = [[0, 1], [2, 3], [4, 5], [6, 7]]

_CACHE = {}


def build_nc():
    from contextlib import ExitStack
    import concourse.mybir as mybir
    import concourse.tile as tile
    from concourse import bacc

    F32 = mybir.dt.float32
    FR = mybir.dt.float32r
    BF = mybir.dt.bfloat16
    AF = mybir.ActivationFunctionType

    nc = bacc.Bacc("TRN2", debug=False, num_devices=NCORES)

    xT = nc.dram_tensor("xT", (D, NQ), BF, kind="ExternalInput")   # own rows only
    wqT = nc.dram_tensor("wqT", (D, D), BF, kind="ExternalInput")
    wkT = nc.dram_tensor("wkT", (D, D), BF, kind="ExternalInput")
    wvT = nc.dram_tensor("wvT", (D, D), BF, kind="ExternalInput")
    bq = nc.dram_tensor("bq", (D,), F32, kind="ExternalInput")
    bk = nc.dram_tensor("bk", (D,), F32, kind="ExternalInput")
    bv = nc.dram_tensor("bv", (D,), F32, kind="ExternalInput")
    out = nc.dram_tensor("out", (NQ, D), F32, kind="ExternalOutput")

    ET = D // P            # 8 e-tiles
    DT = D // P            # 8 d-tiles
    HC = NQ // 512         # 2 s-chunks over own half
    SB = S // P            # 16 j-tiles (full seq)
    HB = NQ // P           # 8 j-tiles (own half)
    IG = NQ // 512         # 2 i-chunks
    EC = D // 512          # 2 e-chunks

    with tile.TileContext(nc) as tc, ExitStack() as ctx:
        consts = ctx.enter_context(tc.tile_pool(name="consts", bufs=1))

        _eng = [nc.sync, nc.scalar]
        _dmac = [0]
        def dma(out_ap, in_ap):
            e = _eng[_dmac[0] % len(_eng)]
            _dmac[0] += 1
            e.dma_start(out_ap, in_ap)

        # resident tensors
        kqv = ctx.enter_context(tc.tile_pool(name="kqv", bufs=1))
        kT_r = [kqv.tile([P, ET, NQ], BF, tag=f"kT{r}", name=f"kT{r}")
                for r in range(2)]
        qT = kqv.tile([P, ET, NQ], BF)      # [d-part, e-tile, i]
        vN_rr = [[kqv.tile([P, 4, D], BF, tag=f"vN{r}_{s}", name=f"vN{r}_{s}")
                  for s in range(2)] for r in range(2)]
        bv_bcast = consts.tile([P, D], F32)

        # DRAM bounce buffers for the pairwise K/V AllGather
        ccd = ctx.enter_context(tc.tile_pool(name="ccd", bufs=1, space="DRAM"))
        kb_in = [ccd.tile([P, ET, 512], BF, tag=f"kbi{s}", name=f"kbi{s}")
                 for s in range(2)]
        kb_out = [ccd.tile([2, P, ET, 512], BF, tag=f"kbo{s}", name=f"kbo{s}")
                  for s in range(2)]
        vb_in = [ccd.tile([P, 4, D], BF, tag=f"vbi{s}", name=f"vbi{s}")
                 for s in range(2)]
        vb_out = [ccd.tile([2, P, 4, D], BF, tag=f"vbo{s}", name=f"vbo{s}")
                  for s in range(2)]

        # ---- Phase 1: projections over own half, single pass over x ----
        with tc.tile_pool(name="w", bufs=1) as wp, \
             tc.tile_pool(name="xc", bufs=2) as xcp, \
             tc.tile_pool(name="kv_own", bufs=1) as ownp:
            kown_c = [ownp.tile([P, ET, 512], BF, tag=f"ko{c}", name=f"ko{c}")
                      for c in range(HC)]   # [d-part, e-tile, own j] per chunk
            vown_c = [ownp.tile([P, 4, D], BF, tag=f"vo{c}", name=f"vo{c}")
                      for c in range(HC)]   # [own j-part, j-tile, e] per chunk
            wk_sb = wp.tile([P, DT, D], BF)
            wq_sb = wp.tile([P, DT, D], BF)
            wv_sb = wp.tile([P, DT, D], BF)
            # DMA issue in consumption order: wk+x first, then wv, wq
            xc = []
            for hc in range(HC):
                xc.append(xcp.tile([P, DT, 512], BF, tag="xc", name=f"xc{hc}"))
            for dt in range(DT):
                dma(wk_sb[:, dt, :], wkT[dt * P:(dt + 1) * P, :])
                dma(xc[0][:, dt, :], xT[dt * P:(dt + 1) * P, 0:512])

            for dt in range(DT):
                dma(xc[1][:, dt, :], xT[dt * P:(dt + 1) * P, 512:1024])

            # constants (issued after the critical-path DMAs)
            ones_f = consts.tile([1, P], F32)
            nc.vector.memset(ones_f[:], 1.0)
            ones = consts.tile([1, P], FR)
            nc.gpsimd.dma_start(ones[:], ones_f[:])
            onesb = consts.tile([P, 1], F32)
            nc.vector.memset(onesb[:], 1.0)
            bv_sb = consts.tile([1, D], FR)
            nc.gpsimd.dma_start(bv_sb[:], bv[:].rearrange("(one d) -> one d", one=1))
            bq_sb = consts.tile([P, ET], F32)
            nc.scalar.dma_start(bq_sb[:], bq[:].rearrange("(t p) -> p t", p=P))
            bk_sb = consts.tile([P, ET], F32)
            nc.scalar.dma_start(bk_sb[:], bk[:].rearrange("(t p) -> p t", p=P))
            for dt in range(DT):
                dma(wv_sb[:, dt, :], wvT[dt * P:(dt + 1) * P, :])
            for dt in range(DT):
                dma(wq_sb[:, dt, :], wqT[dt * P:(dt + 1) * P, :])

            # chunk-0 K projection dt-outer/et-inner: the first 8 matmuls only
            # need wk[dt0]+x0[dt0], so the PE starts as soon as ~384KB landed.
            with tc.tile_pool(name="p8", bufs=8, space="PSUM") as p8:
                psk0 = [p8.tile([P, 512], F32, tag="p8", name=f"psk0_{et}")
                        for et in range(ET)]
                for dt in range(DT):
                    for et in range(ET):
                        nc.tensor.matmul(psk0[et][:],
                                         wk_sb[:, dt, et * P:(et + 1) * P],
                                         xc[0][:, dt, :], start=(dt == 0),
                                         stop=(dt == DT - 1))
                for et in range(ET):
                    nc.vector.tensor_scalar_add(kown_c[0][:, et, :],
                                                psk0[et][:], bk_sb[:, et:et + 1])

            with tc.tile_pool(name="pp", bufs=3, space="PSUM") as pp:
                # K chunk 0 complete -> bounce + first AllGather piece NOW
                for et in range(ET):
                    dma(kb_in[0][:, et, :], kown_c[0][:, et, :])
                nc.gpsimd.collective_compute(
                    "AllGather", mybir.AluOpType.bypass, replica_groups=PAIRS,
                    ins=[kb_in[0][:].opt()], outs=[kb_out[0][:].opt()])
                for r in range(2):
                    for et in range(ET):
                        dma(kT_r[r][:, et, 0:512], kb_out[0][r, :, et, :])
                # K chunk 1
                for et in range(ET):
                    psk = pp.tile([P, 512], F32, tag="pp")
                    for dt in range(DT):
                        nc.tensor.matmul(psk[:], wk_sb[:, dt, et * P:(et + 1) * P],
                                         xc[1][:, dt, :], start=(dt == 0),
                                         stop=(dt == DT - 1))
                    nc.vector.tensor_scalar_add(kown_c[1][:, et, :],
                                                psk[:], bk_sb[:, et:et + 1])
                # K chunk 1 -> second AllGather piece
                for et in range(ET):
                    dma(kb_in[1][:, et, :], kown_c[1][:, et, :])
                nc.gpsimd.collective_compute(
                    "AllGather", mybir.AluOpType.bypass, replica_groups=PAIRS,
                    ins=[kb_in[1][:].opt()], outs=[kb_out[1][:].opt()])
                for r in range(2):
                    for et in range(ET):
                        dma(kT_r[r][:, et, 512:1024], kb_out[1][r, :, et, :])

                # bv broadcast to [P, D] via ones.T @ bv (K=1 matmul)
                for ec in range(EC):
                    pstmp = pp.tile([P, 512], F32, tag="pp")
                    nc.tensor.matmul(pstmp[:], ones[:],
                                     bv_sb[:, ec * 512:(ec + 1) * 512],
                                     start=True, stop=True)
                    nc.any.tensor_copy(bv_bcast[:, ec * 512:(ec + 1) * 512], pstmp[:])

                # V projection (own half)
                for hc in range(HC):
                    for sb_i in range(4):
                        jg = hc * 4 + sb_i
                        for ec in range(EC):
                            psv = pp.tile([P, 512], F32, tag="pp")
                            for dt in range(DT):
                                nc.tensor.matmul(psv[:],
                                                 xc[hc][:, dt, sb_i * P:(sb_i + 1) * P],
                                                 wv_sb[:, dt, ec * 512:(ec + 1) * 512],
                                                 start=(dt == 0), stop=(dt == DT - 1))
                            nc.any.tensor_copy(
                                vown_c[hc][:, sb_i, ec * 512:(ec + 1) * 512], psv[:])
                # V-half complete -> bounce out + AllGather (overlaps Q)
                for jg in range(HB):
                    dma(vb_in[jg // 4][:, jg % 4, :], vown_c[jg // 4][:, jg % 4, :])
                for s in range(2):
                    nc.gpsimd.collective_compute(
                        "AllGather", mybir.AluOpType.bypass, replica_groups=PAIRS,
                        ins=[vb_in[s][:].opt()], outs=[vb_out[s][:].opt()])
                    for r in range(2):
                        for jg in range(4):
                            dma(vN_rr[r][s][:, jg, :], vb_out[s][r, :, jg, :])

                # Q projection
                for hc in range(HC):
                    for et in range(ET):
                        psq = pp.tile([P, 512], F32, tag="pp")
                        for dt in range(DT):
                            nc.tensor.matmul(psq[:],
                                             wq_sb[:, dt, et * P:(et + 1) * P],
                                             xc[hc][:, dt, :], start=(dt == 0),
                                             stop=(dt == DT - 1))
                        nc.vector.tensor_scalar_add(
                            qT[:, et, hc * 512:(hc + 1) * 512],
                            psq[:], bq_sb[:, et:et + 1])

        # ---- Phase 2: attention ----
        # All score chunks first (attn for both i-chunks stays resident), then
        # all output matmuls: pushes the first vN consumer ~30us later so the
        # V AllGather + completion-polling latency is fully hidden. The scores
        # PSUM pools close before the out phase so outps can take all 8 banks,
        # keeping 4 output blocks in flight ahead of the epilogue drains.
        with tc.tile_pool(name="attn", bufs=1) as attnp, \
             tc.tile_pool(name="epi2", bufs=2) as epi2p:
            attnTs = [attnp.tile([P, SB, 512], BF, tag=f"attnT{g}", name=f"attnT{g}")
                      for g in range(IG)]
            invss = []
            with tc.tile_pool(name="scps", bufs=2, space="PSUM") as scps, \
                 tc.tile_pool(name="rsps", bufs=2, space="PSUM") as rsps, \
                 tc.tile_pool(name="rsdram", bufs=2, space="DRAM") as rsdram, \
                 tc.tile_pool(name="epi", bufs=2) as epip:
                for g in range(IG):
                    attnT = attnTs[g]
                    # accumulate sum over j-tiles on DVE (idle during scores);
                    # the partition sum is then ONE ones-matmul instead of 16
                    acc = epip.tile([P, 512], F32, tag="acc", name=f"acc{g}")
                    rs = rsps.tile([1, 512], F32, tag="rs", name=f"rs{g}")
                    for jt in range(SB):
                        sc_ps = scps.tile([P, 512], F32, tag="scps")
                        for et in range(ET):
                            nc.tensor.matmul(sc_ps[:],
                                             kT_r[jt // HB][:, et, (jt % HB) * P:(jt % HB + 1) * P],
                                             qT[:, et, g * 512:(g + 1) * 512],
                                             start=(et == 0), stop=(et == ET - 1))
                        nc.scalar.activation(attnT[:, jt, :], sc_ps[:], AF.Exp,
                                             scale=INV_SQRT_D)
                        if jt == 0:
                            nc.vector.tensor_copy(acc[:], attnT[:, 0, :])
                        else:
                            nc.vector.tensor_add(acc[:], acc[:], attnT[:, jt, :])
                    nc.tensor.matmul(rs[:], onesb[:], acc[:], start=True, stop=True)
                    # rowsums -> per-partition [128, 4] via DRAM bounce
                    rs_sb = epip.tile([1, 512], F32, tag="rs_sb")
                    nc.vector.tensor_copy(rs_sb[:], rs[:])
                    rs_d = rsdram.tile([1, 512], F32, tag="rs_d")
                    nc.sync.dma_start(rs_d[:], rs_sb[:])
                    rsT = epip.tile([P, 4], F32, tag="rsT")
                    nc.sync.dma_start(
                        rsT[:], rs_d[:].rearrange("one (b p) -> p (one b)", p=P))
                    invs = epi2p.tile([P, 4], F32, tag="invs", name=f"invs{g}")
                    nc.vector.reciprocal(invs[:], rsT[:])
                    invss.append(invs)
            # output matmuls + epilogue per 128-query block
            with tc.tile_pool(name="outps", bufs=4, space="PSUM") as outps:
                for g in range(IG):
                    attnT = attnTs[g]
                    invs = invss[g]
                    for ib in range(4):
                        i0 = ib * P
                        ops = [outps.tile([P, 512], F32, tag=f"outps{ec}",
                                          name=f"ops{g}_{ib}_{ec}") for ec in range(EC)]
                        for jt in range(SB):
                            for ec in range(EC):
                                nc.tensor.matmul(ops[ec][:],
                                                 attnT[:, jt, i0:i0 + P],
                                                 vN_rr[jt // HB][(jt % HB) // 4][:, jt % 4, ec * 512:(ec + 1) * 512],
                                                 start=(jt == 0), stop=(jt == SB - 1))
                        out_sb = epi2p.tile([P, D], F32, tag="out_sb")
                        # per-half pipeline: ACT normalize -> DVE bias -> DMA,
                        # so the final block's drain chain is ~half as long
                        r0 = g * 512 + i0
                        for ec in range(EC):
                            sl = slice(ec * 512, (ec + 1) * 512)
                            nc.scalar.activation(out_sb[:, sl], ops[ec][:],
                                                 AF.Copy, scale=invs[:, ib:ib + 1])
                            nc.vector.tensor_add(out_sb[:, sl], out_sb[:, sl],
                                                 bv_bcast[:, sl])
                            dma(out[r0:r0 + P, sl], out_sb[:, sl])

    nc.compile()
    return nc


def make_in_maps(x, Wq, bq, Wk, bk, Wv, bv):
    import ml_dtypes
    BF = ml_dtypes.bfloat16
    x = np.asarray(x, np.float32)
    wqT = np.ascontiguousarray(np.asarray(Wq, np.float32).T.astype(BF))
    wkT = np.ascontiguousarray(np.asarray(Wk, np.float32).T.astype(BF))
    wvT = np.ascontiguousarray(np.asarray(Wv, np.float32).T.astype(BF))
    bq = np.ascontiguousarray(np.asarray(bq, np.float32))
    bk = np.ascontiguousarray(np.asarray(bk, np.float32))
    bv = np.ascontiguousarray(np.asarray(bv, np.float32))
    in_maps = []
    for c in range(NCORES):
        b, h = c // 2, c % 2
        xb = x[b]
        in_maps.append({
            "xT": np.ascontiguousarray(xb[h * NQ:(h + 1) * NQ].T.astype(BF)),
            "wqT": wqT, "wkT": wkT, "wvT": wvT,
            "bq": bq, "bk": bk, "bv": bv,
        })
    return in_maps


def get_nc():
    if "nc" not in _CACHE:
        _CACHE["nc"] = build_nc()
    return _CACHE["nc"]


def kernel(x, Wq, bq, Wk, bk, Wv, bv):
    from concourse.bass_utils import run_bass_kernel_spmd
    nc = get_nc()
    in_maps = make_in_maps(x, Wq, bq, Wk, bk, Wv, bv)
    res = run_bass_kernel_spmd(nc, in_maps, core_ids=list(range(NCORES)))
    out = np.empty((B, S, D), np.float32)
    for c in range(NCORES):
        b, h = c // 2, c % 2
        out[b, h * NQ:(h + 1) * NQ] = res.results[c]["out"]
    return out
